# revision 32
# baseline (speedup 1.0000x reference)
"""Trainium2 Bass kernel for Bahdanau-style additive attention.

reference math (per batch b):
    W_enc = enc @ W_a                      # [Te, D]
    U_dec = dec @ U_a                      # [Td, D]
    e     = tanh(W_enc[None,:,:] + U_dec[:,None,:])   # [Td, Te, D]
    score = e @ V_a                        # [Td, Te]
    attn  = softmax(score, axis=-1)        # [Td, Te]
    ctx   = attn @ enc                     # [Td, D]
returns (ctx, attn).

Sharding: data-parallel over (b, Td-half) -> 8 cores, no collectives.

Per-core pipeline (d on partitions for the elementwise phase):
  - PE transposes + bf16 projections give W_encT [d'(part), te] and
    U_decT [d'(part), t]; first tanh group is te-halved so ACT starts
    as early as possible.
  - elementwise split to balance engines: most decoder positions run
    e_in = W_encT + u_t (DVE tensor_scalar, per-partition scalar) then
    tanh on ACT in batched ACTIVATEs (~0.9ns/elem, the ~105us wall);
    the rest use tanh(x) = 1 - 2/(1 + e^{2w} e^{2u}) entirely on DVE
    (fused multiply-add + reciprocal_approx_fast).
  - V-dot in ROW form: V is the 1-column stationary operand, e streams
    as the moving operand (1 col/cycle; the stationary-e alternative
    pays ~2x in LDWEIGHTS). Four positions share one PSUM bank at
    partitions {0,32,64,96} via tile_position col-groups (V broadcast
    across 32 stationary columns keeps the bank initialized), so one
    97-lane DVE copy evacuates 4 score rows at free-size cost.
  - scores repack to [t, te] with contiguous-block DMAs (bank g holds
    t = 32j + g), then exp with fused accum_out denominator, bf16 PE
    transposes + bf16 context matmul, and per-partition normalization.

Measured on trn2 (8 cores): 145.2us, rel err ~3e-3 (bf16-dominated).
"""

import numpy as np

import concourse.bass as bass
import concourse.bacc as bacc
import concourse.tile as tile
from concourse import mybir
from concourse.bass_utils import run_bass_kernel_spmd
from concourse.masks import make_identity

F32 = mybir.dt.float32
BF16 = mybir.dt.bfloat16

B, Te, Td, D = 4, 512, 256, 256
N_CORES = 8
TDS = Td * B // N_CORES          # 128 decoder positions per core
NB_TE = Te // 128                # 4 te blocks
NB_D = D // 128                  # 2 d blocks
G = 4                            # decoder positions per tanh batch
# ACT/DVE split: t < TANH_T run tanh on ACT; the rest use
# tanh(x) = 1 - 2/(1 + e^{2w} e^{2u}) with the fused multiply-add +
# reciprocal on DVE, balancing the two engines (ACT is otherwise the
# ~134us bottleneck while DVE sits at ~50us).
TANH_T = 116
NGA = TANH_T // G
WORK_BUFS = 4

_NC = None


def _build_core_graph():
    # Bacc (not plain Bass): its compile() runs move_matmul_waits_to_ldweights
    # + generate_event_semaphores, which legalize to <=1 sync wait per
    # instruction (walrus rejects multi-wait Matmults with "Too many sync
    # wait commands").
    nc = bacc.Bacc(trn_type="TRN2")
    enc = nc.dram_tensor("enc", [Te, D], F32, kind="ExternalInput")
    dec = nc.dram_tensor("dec", [TDS, D], F32, kind="ExternalInput")
    w_a = nc.dram_tensor("w_a", [D, D], F32, kind="ExternalInput")
    u_a = nc.dram_tensor("u_a", [D, D], F32, kind="ExternalInput")
    v_a = nc.dram_tensor("v_a", [D, 1], F32, kind="ExternalInput")
    out = nc.dram_tensor("out", [TDS, D + Te], F32, kind="ExternalOutput")

    with tile.TileContext(nc) as tc:
        _body(tc, enc, dec, w_a, u_a, v_a, out)
    nc.compile()
    return nc


def _body(tc, enc, dec, w_a, u_a, v_a, out):
    nc = tc.nc
    from contextlib import ExitStack

    with ExitStack() as ctx:
        const = ctx.enter_context(tc.tile_pool(name="const", bufs=1))
        work = ctx.enter_context(tc.tile_pool(name="work", bufs=WORK_BUFS))
        pbig = ctx.enter_context(
            tc.tile_pool(name="pbig", bufs=2, space=bass.MemorySpace.PSUM)
        )
        pbank = ctx.enter_context(
            tc.tile_pool(name="pbank", bufs=3, space=bass.MemorySpace.PSUM)
        )
        recpool = ctx.enter_context(
            tc.tile_pool(name="recpool", bufs=max(1, TDS - TANH_T))
        )

        ident = const.tile([128, 128], F32)
        make_identity(nc, ident[:])

        # ---- inputs -> SBUF
        # enc_ext carries a leading ones column per te-block so the context
        # matmul also produces the softmax denominator in column 0.
        enc_ext = const.tile([128, NB_TE, 1 + D], F32)
        nc.vector.memset(enc_ext[:, :, 0:1], 1.0)
        enc_r = enc.rearrange("(n p) d -> p n d", p=128)
        for n in range(NB_TE):
            # split across DMA queues so the transposes can start sooner
            nc.sync.dma_start(out=enc_ext[:, n, 1:], in_=enc_r[:, n, :])
        dec_sb = const.tile([128, D], F32)
        nc.sync.dma_start(out=dec_sb[:], in_=dec[:, :])
        wa_sb = const.tile([128, NB_D, D], F32)
        nc.sync.dma_start(out=wa_sb[:], in_=w_a.rearrange("(k p) e -> p k e", p=128))
        ua_sb = const.tile([128, NB_D, D], F32)
        nc.sync.dma_start(out=ua_sb[:], in_=u_a.rearrange("(k p) e -> p k e", p=128))
        v_sb = const.tile([128, NB_D], F32)
        nc.sync.dma_start(out=v_sb[:], in_=v_a.rearrange("(k p) o -> p (k o)", p=128))
        v_bf = const.tile([128, NB_D], BF16)
        nc.vector.tensor_copy(v_bf[:], v_sb[:])

        # ---- bf16 working copies (prep-latency: bf16 matmuls stream 4x
        # faster than f32, and the first tanh group is chained behind the
        # enc DMA -> transpose -> projection path)
        ident_bf = const.tile([128, 128], BF16)
        nc.vector.tensor_copy(ident_bf[:], ident[:])
        encb = const.tile([128, NB_TE, D], BF16)
        for h in range(2):
            nc.vector.tensor_copy(
                encb[:, 2 * h : 2 * (h + 1), :], enc_ext[:, 2 * h : 2 * (h + 1), 1:]
            )
        wab = const.tile([128, NB_D, D], BF16)
        nc.vector.tensor_copy(wab[:], wa_sb[:])
        uab = const.tile([128, NB_D, D], BF16)
        nc.vector.tensor_copy(uab[:], ua_sb[:])
        decb = const.tile([128, D], BF16)
        nc.vector.tensor_copy(decb[:], dec_sb[:])

        # ---- encT [d(part), te], decT [d(part), t] via PE transposes (bf16)
        encT = const.tile([128, NB_D, Te], BF16)
        for n in range(NB_TE):
            for k in range(NB_D):
                tp = pbig.tile([128, 128], BF16, tag="tr")
                nc.tensor.transpose(
                    tp[:], encb[:, n, 128 * k : 128 * (k + 1)], ident_bf[:]
                )
                nc.vector.tensor_copy(encT[:, k, 128 * n : 128 * (n + 1)], tp[:])
        decT = const.tile([128, NB_D, TDS], BF16)
        for k in range(NB_D):
            tp = pbig.tile([128, 128], BF16, tag="tr")
            nc.tensor.transpose(tp[:], decb[:, 128 * k : 128 * (k + 1)], ident_bf[:])
            nc.vector.tensor_copy(decT[:, k, :], tp[:])

        # ---- projections (PE, bf16 operands, f32 PSUM), te-halved so the
        # first half of w_encT lands as early as possible
        w_encT = const.tile([128, NB_D, Te], BF16)
        HT = Te // 2
        for h in range(2):
            for j in range(NB_D):
                pj = pbig.tile([128, HT], F32, tag="proj")
                for k in range(NB_D):
                    nc.tensor.matmul(
                        pj[:],
                        wab[:, k, 128 * j : 128 * (j + 1)],
                        encT[:, k, HT * h : HT * (h + 1)],
                        start=(k == 0),
                        stop=(k == NB_D - 1),
                    )
                nc.scalar.copy(w_encT[:, j, HT * h : HT * (h + 1)], pj[:])
        # scalar operand of tensor_scalar must be f32 -> keep u_decT f32
        u_decT = const.tile([128, NB_D, TDS], F32)
        for j in range(NB_D):
            pj = pbig.tile([128, HT], F32, tag="proj")
            for k in range(NB_D):
                nc.tensor.matmul(
                    pj[:, :TDS],
                    uab[:, k, 128 * j : 128 * (j + 1)],
                    decT[:, k, :],
                    start=(k == 0),
                    stop=(k == NB_D - 1),
                )
            nc.scalar.copy(u_decT[:, j, :], pj[:, :TDS])

        # P2 for the DVE tanh-identity path, sourced from bf16 w_encT
        # (emitted after the first tanh group so it does not delay it)
        p2 = const.tile([128, NB_D, Te], F32)
                # Q2[j][d', t] = exp(2 * u_decT) (f32; tensor_scalar scalars must be
        # f32). -2*V feeds the identity-path matmuls: score' = -2 * sum V*r
        # differs from the true score by a per-t constant, which softmax
        # normalizes away.
        q2 = const.tile([128, NB_D, TDS], F32)
        v2m = const.tile([128, NB_D], F32)
        nc.vector.tensor_scalar_mul(v2m[:], v_sb[:], -2.0)

        # ---- main loop (v5): row-form V-dot.
        # The v4 stationary-e form streamed all of e through the PE weight
        # port (LDWEIGHTS ~196us on HW). Row form keeps V stationary (1-col
        # LDW) and streams e as the moving operand (1 col/cycle @2.4GHz,
        # ~55us). Scores come out as [1, 512] rows; we pack 4 decoder
        # positions per PSUM bank at partitions {0,32,64,96} via
        # tile_position col-groups, evacuate each bank with ONE 97-lane DVE
        # copy (cost is free-size-bound, so 4 rows for the price of one),
        # and repack to [t, te] later with 4 strided DMAs.
        NGB = TDS // 4  # bank-groups; bank g holds t = 32j + g at partition 32j
        stage = const.tile([128, NGB, Te], BF16)

        def vdot_row(bank, j, src_e, vcol):
            # M=32 with V broadcast across the stationary columns: all 32
            # partitions of the col-group get (duplicate) score rows, which
            # keeps the bank fully initialized for the 97-lane evacuation
            # copy. Streaming cost is N-bound, so the extra rows are free.
            for k in range(NB_D):
                nc.tensor.matmul(
                    bank[32 * j : 32 * j + 32, :],
                    vcol[:, k : k + 1].to_broadcast((128, 32)),
                    src_e[:, k, :],
                    start=(k == 0),
                    stop=(k == NB_D - 1),
                    tile_position=(0, 32 * j),
                )

        def exp_dve(t):
            den = work.tile([128, NB_D, Te], F32, tag="den")
            for k in range(NB_D):
                nc.gpsimd.tensor_scalar(
                    den[:, k, :],
                    p2[:, k, :],
                    q2[:, k, t : t + 1],
                    1.0,
                    op0=mybir.AluOpType.mult,
                    op1=mybir.AluOpType.add,
                )
            # den = 1 + e^{2(w+u)} in [1, ~1e6]: safely inside
            # reciprocal_approx_fast's domain.
            rec = recpool.tile([128, NB_D, Te], F32, tag="rec")
            nc.vector.reciprocal_approx_fast(out=rec[:], in_=den[:])
            return rec

        packed = const.tile([128, Te], BF16)
        HS = NGB // 2
        exp_gs = list(range(NGA, NGB))
        exp_ts = [32 * j + g for g in exp_gs for j in range(4)]
        recs = {}
        # produce all identity-path reciprocals during the first ~2/3 of the
        # tanh groups, and consume each exp bank mid-loop so its f32 V-dots
        # (4 cyc/col on PE) hide in PE idle time instead of extending the
        # tail.
        DVE_GS = max(1, (2 * NGA) // 3)
        consume_at = {}
        for idx, g in enumerate(exp_gs):
            consume_at.setdefault(DVE_GS + 2 + 3 * idx, []).append(g)

        def do_exp_bank(g):
            bank = pbank.tile([128, Te], F32, tag="bank")
            for j in range(4):
                vdot_row(bank, j, recs[32 * j + g], v2m)
            nc.vector.tensor_copy(stage[0:97, g, :], bank[0:97, :])

        for g in range(NGA):
            e_in = work.tile([128, G, NB_D, Te], BF16, tag="e_in")
            e_tanh = work.tile([128, G, NB_D, Te], BF16, tag="e_tanh")
            if g == 0:
                # First group split into te-halves so the first TANH starts
                # as soon as the first half of w_encT exists; P2/Q2 for the
                # identity path are emitted right after (ACT stays the
                # bottleneck either way, but they must not delay this tanh).
                for h in range(2):
                    for i in range(G):
                        t = 32 * i + g
                        for k in range(NB_D):
                            nc.vector.tensor_scalar_add(
                                e_in[:, i, k, HT * h : HT * (h + 1)],
                                w_encT[:, k, HT * h : HT * (h + 1)],
                                u_decT[:, k, t : t + 1],
                            )
                    nc.scalar.activation(
                        e_in[:, :, :, HT * h : HT * (h + 1)],
                        e_in[:, :, :, HT * h : HT * (h + 1)],
                        mybir.ActivationFunctionType.Tanh,
                    )
                e_tanh = e_in
                nc.scalar.activation(
                    p2[:], w_encT[:], mybir.ActivationFunctionType.Exp, scale=2.0
                )
                nc.scalar.activation(
                    q2[:], u_decT[:], mybir.ActivationFunctionType.Exp, scale=2.0
                )
            else:
                for i in range(G):
                    t = 32 * i + g
                    for k in range(NB_D):
                        nc.vector.tensor_scalar_add(
                            e_in[:, i, k, :], w_encT[:, k, :], u_decT[:, k, t : t + 1]
                        )
                nc.scalar.activation(
                    e_tanh[:], e_in[:], mybir.ActivationFunctionType.Tanh
                )
            bank = pbank.tile([128, Te], F32, tag="bank")
            for i in range(G):
                vdot_row(bank, i, e_tanh[:, i], v_bf)
            nc.vector.tensor_copy(stage[0:97, g, :], bank[0:97, :])
            lo = len(exp_ts) * g // DVE_GS
            hi = len(exp_ts) * (g + 1) // DVE_GS
            for t in exp_ts[lo:hi]:
                recs[t] = exp_dve(t)
            for eg in consume_at.get(g, []):
                do_exp_bank(eg)
            if g == HS - 1:
                # first-half repack can start as soon as slots 0..HS-1 exist
                for j in range(4):
                    nc.sync.dma_start(
                        out=packed[32 * j : 32 * j + HS, :],
                        in_=stage[32 * j : 32 * j + 1, 0:HS, :],
                    )
        # any exp banks whose consume slot fell past the last group
        for k2, es in consume_at.items():
            if k2 >= NGA:
                for eg in es:
                    do_exp_bank(eg)

        # repack staging (t = 32j + g at partition 32j, slot g) to [t, te]:
        # each j gives one contiguous 32-row block
        for j in range(4):
            nc.sync.dma_start(
                out=packed[32 * j + HS : 32 * j + 2 * HS, :],
                in_=stage[32 * j : 32 * j + 1, HS : 2 * HS, :],
            )

        # ---- softmax + context
        # scores are bounded by sum |V| (~10), so exp without max-subtraction
        # is safe in f32. accum_out gives the softmax denominator for free.
        w_s = const.tile([128, Te], BF16)
        denom = const.tile([128, 1], F32)
        nc.scalar.activation(
            w_s[:], packed[:], mybir.ActivationFunctionType.Exp, accum_out=denom[:]
        )
        rdenom = const.tile([128, 1], F32)
        nc.vector.reciprocal(rdenom[:], denom[:])

        # wT for the context matmul via PE transposes of the packed weights
        wT = const.tile([128, NB_TE, 128], BF16)
        for c in range(NB_TE):
            tp = pbig.tile([128, 128], BF16, tag="tr")
            nc.tensor.transpose(tp[:], w_s[:, 128 * c : 128 * (c + 1)], ident_bf[:])
            nc.vector.tensor_copy(wT[:, c, :], tp[:])
        ctxp = pbig.tile([128, D], F32, tag="proj")
        for c in range(NB_TE):
            nc.tensor.matmul(
                ctxp[:],
                wT[:, c, :],
                encb[:, c, :],
                start=(c == 0),
                stop=(c == NB_TE - 1),
            )
        ctx_sb = const.tile([128, D], F32)
        nc.vector.tensor_scalar_mul(ctx_sb[:], ctxp[:], rdenom[:])
        nc.sync.dma_start(out=out[:, 0:D], in_=ctx_sb[:])

        attn = const.tile([128, Te], F32)
        nc.vector.tensor_scalar_mul(attn[:], w_s[:], rdenom[:])
        nc.sync.dma_start(out=out[:, D:], in_=attn[:])


def _get_nc():
    global _NC
    if _NC is None:
        _NC = _build_core_graph()
    return _NC


def _make_in_maps(inputs):
    enc_full = np.ascontiguousarray(np.asarray(inputs["encoder_output"], np.float32))
    dec_full = np.ascontiguousarray(np.asarray(inputs["decoder_output"], np.float32))
    w_a = np.ascontiguousarray(np.asarray(inputs["W_a"], np.float32))
    u_a = np.ascontiguousarray(np.asarray(inputs["U_a"], np.float32))
    v_a = np.ascontiguousarray(np.asarray(inputs["V_a"], np.float32))
    in_maps = []
    for c in range(N_CORES):
        b, h = divmod(c, Td // TDS)
        in_maps.append(
            {
                "enc": enc_full[b],
                "dec": np.ascontiguousarray(dec_full[b, h * TDS : (h + 1) * TDS]),
                "w_a": w_a,
                "u_a": u_a,
                "v_a": v_a,
            }
        )
    return in_maps


def _assemble(results):
    ctx = np.empty((B, Td, D), np.float32)
    attn = np.empty((B, Td, Te), np.float32)
    for c in range(N_CORES):
        o = np.asarray(results[c]["out"])
        b, h = divmod(c, Td // TDS)
        ctx[b, h * TDS : (h + 1) * TDS] = o[:, :D]
        attn[b, h * TDS : (h + 1) * TDS] = o[:, D:]
    return ctx, attn


def _run(inputs, trace=False, **kwargs):
    nc = _get_nc()
    in_maps = _make_in_maps(inputs)
    res = run_bass_kernel_spmd(
        nc, in_maps, core_ids=list(range(N_CORES)), trace=trace, **kwargs
    )
    return _assemble(res.results), res


def kernel(**inputs):
    return _run(inputs)[0]


# revision 34
# speedup vs baseline: 1.0190x; 1.0190x over previous
"""Trainium2 Bass kernel for Bahdanau-style additive attention.

reference math (per batch b):
    W_enc = enc @ W_a                      # [Te, D]
    U_dec = dec @ U_a                      # [Td, D]
    e     = tanh(W_enc[None,:,:] + U_dec[:,None,:])   # [Td, Te, D]
    score = e @ V_a                        # [Td, Te]
    attn  = softmax(score, axis=-1)        # [Td, Te]
    ctx   = attn @ enc                     # [Td, D]
returns (ctx, attn).

Sharding: data-parallel over (b, Td-half) -> 8 cores, no collectives.

Per-core pipeline (d on partitions for the elementwise phase):
  - PE transposes + bf16 projections give W_encT [d'(part), te] and
    U_decT [d'(part), t]; first tanh group is te-halved so ACT starts
    as early as possible.
  - elementwise split to balance engines: most decoder positions run
    e_in = W_encT + u_t (DVE tensor_scalar, per-partition scalar) then
    tanh on ACT in batched ACTIVATEs (~0.9ns/elem, the ~105us wall);
    the rest use tanh(x) = 1 - 2/(1 + e^{2w} e^{2u}) entirely on DVE
    (fused multiply-add + reciprocal_approx_fast).
  - V-dot in ROW form: V is the 1-column stationary operand, e streams
    as the moving operand (1 col/cycle; the stationary-e alternative
    pays ~2x in LDWEIGHTS). Four positions share one PSUM bank at
    partitions {0,32,64,96} via tile_position col-groups (V broadcast
    across 32 stationary columns keeps the bank initialized), so one
    97-lane DVE copy evacuates 4 score rows at free-size cost.
  - scores repack to [t, te] with contiguous-block DMAs (bank g holds
    t = 32j + g), then exp with fused accum_out denominator, bf16 PE
    transposes + bf16 context matmul, and per-partition normalization.

Measured on trn2 (8 cores): 145.2us, rel err ~3e-3 (bf16-dominated).
"""

import numpy as np

import concourse.bass as bass
import concourse.bacc as bacc
import concourse.tile as tile
from concourse import mybir
from concourse.bass_utils import run_bass_kernel_spmd
from concourse.masks import make_identity

F32 = mybir.dt.float32
BF16 = mybir.dt.bfloat16

B, Te, Td, D = 4, 512, 256, 256
N_CORES = 8
TDS = Td * B // N_CORES          # 128 decoder positions per core
NB_TE = Te // 128                # 4 te blocks
NB_D = D // 128                  # 2 d blocks
G = 4                            # decoder positions per tanh batch
# ACT/DVE split: t < TANH_T run tanh on ACT; the rest use
# tanh(x) = 1 - 2/(1 + e^{2w} e^{2u}) with the fused multiply-add +
# reciprocal on DVE, balancing the two engines (ACT is otherwise the
# ~134us bottleneck while DVE sits at ~50us).
TANH_T = 116
NGA = TANH_T // G
WORK_BUFS = 4

_NC = None


def _build_core_graph():
    # Bacc (not plain Bass): its compile() runs move_matmul_waits_to_ldweights
    # + generate_event_semaphores, which legalize to <=1 sync wait per
    # instruction (walrus rejects multi-wait Matmults with "Too many sync
    # wait commands").
    nc = bacc.Bacc(trn_type="TRN2")
    enc = nc.dram_tensor("enc", [Te, D], F32, kind="ExternalInput")
    dec = nc.dram_tensor("dec", [TDS, D], F32, kind="ExternalInput")
    w_a = nc.dram_tensor("w_a", [D, D], F32, kind="ExternalInput")
    u_a = nc.dram_tensor("u_a", [D, D], F32, kind="ExternalInput")
    v_a = nc.dram_tensor("v_a", [D, 1], F32, kind="ExternalInput")
    out = nc.dram_tensor("out", [TDS, D + Te], F32, kind="ExternalOutput")

    with tile.TileContext(nc) as tc:
        _body(tc, enc, dec, w_a, u_a, v_a, out)
    nc.compile()
    return nc


def _body(tc, enc, dec, w_a, u_a, v_a, out):
    nc = tc.nc
    from contextlib import ExitStack

    with ExitStack() as ctx:
        const = ctx.enter_context(tc.tile_pool(name="const", bufs=1))
        work = ctx.enter_context(tc.tile_pool(name="work", bufs=WORK_BUFS))
        pbig = ctx.enter_context(
            tc.tile_pool(name="pbig", bufs=2, space=bass.MemorySpace.PSUM)
        )
        pbank = ctx.enter_context(
            tc.tile_pool(name="pbank", bufs=3, space=bass.MemorySpace.PSUM)
        )
        recpool = ctx.enter_context(
            tc.tile_pool(name="recpool", bufs=max(1, TDS - TANH_T))
        )

        ident = const.tile([128, 128], F32)
        make_identity(nc, ident[:])

        # ---- inputs -> SBUF
        # enc_ext carries a leading ones column per te-block so the context
        # matmul also produces the softmax denominator in column 0.
        enc_ext = const.tile([128, NB_TE, 1 + D], F32)
        nc.vector.memset(enc_ext[:, :, 0:1], 1.0)
        enc_r = enc.rearrange("(n p) d -> p n d", p=128)
        for n in range(NB_TE):
            # split across DMA queues so the transposes can start sooner
            nc.sync.dma_start(out=enc_ext[:, n, 1:], in_=enc_r[:, n, :])
        dec_sb = const.tile([128, D], F32)
        nc.sync.dma_start(out=dec_sb[:], in_=dec[:, :])
        wa_sb = const.tile([128, NB_D, D], F32)
        nc.sync.dma_start(out=wa_sb[:], in_=w_a.rearrange("(k p) e -> p k e", p=128))
        ua_sb = const.tile([128, NB_D, D], F32)
        nc.sync.dma_start(out=ua_sb[:], in_=u_a.rearrange("(k p) e -> p k e", p=128))
        v_sb = const.tile([128, NB_D], F32)
        nc.sync.dma_start(out=v_sb[:], in_=v_a.rearrange("(k p) o -> p (k o)", p=128))
        v_bf = const.tile([128, NB_D], BF16)
        nc.vector.tensor_copy(v_bf[:], v_sb[:])

        # ---- bf16 working copies (prep-latency: bf16 matmuls stream 4x
        # faster than f32, and the first tanh group is chained behind the
        # enc DMA -> transpose -> projection path)
        ident_bf = const.tile([128, 128], BF16)
        nc.vector.tensor_copy(ident_bf[:], ident[:])
        encb = const.tile([128, NB_TE, D], BF16)
        for h in range(2):
            nc.vector.tensor_copy(
                encb[:, 2 * h : 2 * (h + 1), :], enc_ext[:, 2 * h : 2 * (h + 1), 1:]
            )
        wab = const.tile([128, NB_D, D], BF16)
        nc.vector.tensor_copy(wab[:], wa_sb[:])
        uab = const.tile([128, NB_D, D], BF16)
        nc.vector.tensor_copy(uab[:], ua_sb[:])
        decb = const.tile([128, D], BF16)
        nc.vector.tensor_copy(decb[:], dec_sb[:])

        # ---- encT [d(part), te], decT [d(part), t] via PE transposes (bf16)
        encT = const.tile([128, NB_D, Te], BF16)
        for n in range(NB_TE):
            for k in range(NB_D):
                tp = pbig.tile([128, 128], BF16, tag="tr")
                nc.tensor.transpose(
                    tp[:], encb[:, n, 128 * k : 128 * (k + 1)], ident_bf[:]
                )
                nc.vector.tensor_copy(encT[:, k, 128 * n : 128 * (n + 1)], tp[:])
        decT = const.tile([128, NB_D, TDS], BF16)
        for k in range(NB_D):
            tp = pbig.tile([128, 128], BF16, tag="tr")
            nc.tensor.transpose(tp[:], decb[:, 128 * k : 128 * (k + 1)], ident_bf[:])
            nc.vector.tensor_copy(decT[:, k, :], tp[:])

        # ---- projections (PE, bf16 operands, f32 PSUM), te-halved so the
        # first half of w_encT lands as early as possible
        w_encT = const.tile([128, NB_D, Te], BF16)
        HT = Te // 2
        for h in range(2):
            for j in range(NB_D):
                pj = pbig.tile([128, HT], F32, tag="proj")
                for k in range(NB_D):
                    nc.tensor.matmul(
                        pj[:],
                        wab[:, k, 128 * j : 128 * (j + 1)],
                        encT[:, k, HT * h : HT * (h + 1)],
                        start=(k == 0),
                        stop=(k == NB_D - 1),
                    )
                nc.scalar.copy(w_encT[:, j, HT * h : HT * (h + 1)], pj[:])
        # scalar operand of tensor_scalar must be f32 -> keep u_decT f32
        u_decT = const.tile([128, NB_D, TDS], F32)
        for j in range(NB_D):
            pj = pbig.tile([128, HT], F32, tag="proj")
            for k in range(NB_D):
                nc.tensor.matmul(
                    pj[:, :TDS],
                    uab[:, k, 128 * j : 128 * (j + 1)],
                    decT[:, k, :],
                    start=(k == 0),
                    stop=(k == NB_D - 1),
                )
            nc.scalar.copy(u_decT[:, j, :], pj[:, :TDS])

        # P2 for the DVE tanh-identity path, sourced from bf16 w_encT
        # (emitted after the first tanh group so it does not delay it)
        p2 = const.tile([128, NB_D, Te], F32)
                # Q2[j][d', t] = exp(2 * u_decT) (f32; tensor_scalar scalars must be
        # f32). -2*V feeds the identity-path matmuls: score' = -2 * sum V*r
        # differs from the true score by a per-t constant, which softmax
        # normalizes away.
        q2 = const.tile([128, NB_D, TDS], F32)
        v2m = const.tile([128, NB_D], F32)
        nc.vector.tensor_scalar_mul(v2m[:], v_sb[:], -2.0)

        # ---- main loop (v5): row-form V-dot.
        # The v4 stationary-e form streamed all of e through the PE weight
        # port (LDWEIGHTS ~196us on HW). Row form keeps V stationary (1-col
        # LDW) and streams e as the moving operand (1 col/cycle @2.4GHz,
        # ~55us). Scores come out as [1, 512] rows; we pack 4 decoder
        # positions per PSUM bank at partitions {0,32,64,96} via
        # tile_position col-groups, evacuate each bank with ONE 97-lane DVE
        # copy (cost is free-size-bound, so 4 rows for the price of one),
        # and repack to [t, te] later with 4 strided DMAs.
        NGB = TDS // 4  # bank-groups; bank g holds t = 32j + g at partition 32j
        stage = const.tile([128, NGB, Te], BF16)

        def vdot_row(bank, j, src_e, vcol, lo=0, hi=Te):
            # M=32 with V broadcast across the stationary columns: all 32
            # partitions of the col-group get (duplicate) score rows, which
            # keeps the bank fully initialized for the 97-lane evacuation
            # copy. Streaming cost is N-bound, so the extra rows are free.
            for k in range(NB_D):
                nc.tensor.matmul(
                    bank[32 * j : 32 * j + 32, lo:hi],
                    vcol[:, k : k + 1].to_broadcast((128, 32)),
                    src_e[:, k, lo:hi],
                    start=(k == 0),
                    stop=(k == NB_D - 1),
                    tile_position=(0, 32 * j),
                )

        def exp_dve(t):
            den = work.tile([128, NB_D, Te], F32, tag="den")
            for k in range(NB_D):
                nc.vector.tensor_scalar(
                    den[:, k, :],
                    p2[:, k, :],
                    q2[:, k, t : t + 1],
                    1.0,
                    op0=mybir.AluOpType.mult,
                    op1=mybir.AluOpType.add,
                )
            # den = 1 + e^{2(w+u)} in [1, ~1e6]: safely inside
            # reciprocal_approx_fast's domain.
            rec = recpool.tile([128, NB_D, Te], F32, tag="rec")
            nc.vector.reciprocal_approx_fast(out=rec[:], in_=den[:])
            return rec

        packed = const.tile([128, Te], BF16)
        HS = NGB // 2
        exp_gs = list(range(NGA, NGB))
        exp_ts = [32 * j + g for g in exp_gs for j in range(4)]
        recs = {}
        # produce all identity-path reciprocals during the first ~2/3 of the
        # tanh groups, and consume each exp bank mid-loop so its f32 V-dots
        # (4 cyc/col on PE) hide in PE idle time instead of extending the
        # tail.
        DVE_GS = max(1, NGA - 4)
        consume_at = {}
        for idx, g in enumerate(exp_gs):
            consume_at.setdefault(NGA - 8 + 3 * idx, []).append(g)

        def do_exp_bank(g):
            bank = pbank.tile([128, Te], F32, tag="bank")
            for j in range(4):
                vdot_row(bank, j, recs[32 * j + g], v2m)
            nc.vector.tensor_copy(stage[0:97, g, :], bank[0:97, :])

        for g in range(NGA):
            e_in = work.tile([128, G, NB_D, Te], BF16, tag="e_in")
            e_tanh = work.tile([128, G, NB_D, Te], BF16, tag="e_tanh")
            if g == 0:
                # First group split into te-halves so the first TANH starts
                # as soon as the first half of w_encT exists; P2/Q2 for the
                # identity path are emitted right after (ACT stays the
                # bottleneck either way, but they must not delay this tanh).
                for h in range(2):
                    for i in range(G):
                        t = 32 * i + g
                        for k in range(NB_D):
                            nc.vector.tensor_scalar_add(
                                e_in[:, i, k, HT * h : HT * (h + 1)],
                                w_encT[:, k, HT * h : HT * (h + 1)],
                                u_decT[:, k, t : t + 1],
                            )
                    nc.scalar.activation(
                        e_in[:, :, :, HT * h : HT * (h + 1)],
                        e_in[:, :, :, HT * h : HT * (h + 1)],
                        mybir.ActivationFunctionType.Tanh,
                    )
                e_tanh = e_in
                nc.scalar.activation(
                    p2[:], w_encT[:], mybir.ActivationFunctionType.Exp, scale=2.0
                )
                nc.scalar.activation(
                    q2[:], u_decT[:], mybir.ActivationFunctionType.Exp, scale=2.0
                )
            elif g == NGA - 1:
                # last group te-halved too: the kernel tail chains off this
                # group's tanh -> V-dots -> evacuation, so finishing the
                # first half early shortens the drain.
                bank = pbank.tile([128, Te], F32, tag="bank")
                for h in range(2):
                    for i in range(G):
                        t = 32 * i + g
                        for k in range(NB_D):
                            nc.vector.tensor_scalar_add(
                                e_in[:, i, k, HT * h : HT * (h + 1)],
                                w_encT[:, k, HT * h : HT * (h + 1)],
                                u_decT[:, k, t : t + 1],
                            )
                    nc.scalar.activation(
                        e_in[:, :, :, HT * h : HT * (h + 1)],
                        e_in[:, :, :, HT * h : HT * (h + 1)],
                        mybir.ActivationFunctionType.Tanh,
                    )
                    for i in range(G):
                        vdot_row(
                            bank, i, e_in[:, i], v_bf, HT * h, HT * (h + 1)
                        )
                nc.vector.tensor_copy(stage[0:97, g, :], bank[0:97, :])
            else:
                for i in range(G):
                    t = 32 * i + g
                    for k in range(NB_D):
                        nc.vector.tensor_scalar_add(
                            e_in[:, i, k, :], w_encT[:, k, :], u_decT[:, k, t : t + 1]
                        )
                nc.scalar.activation(
                    e_tanh[:], e_in[:], mybir.ActivationFunctionType.Tanh
                )
            if g != NGA - 1:
                bank = pbank.tile([128, Te], F32, tag="bank")
                for i in range(G):
                    vdot_row(bank, i, e_tanh[:, i], v_bf)
                nc.vector.tensor_copy(stage[0:97, g, :], bank[0:97, :])
            lo = len(exp_ts) * g // DVE_GS
            hi = len(exp_ts) * (g + 1) // DVE_GS
            for t in exp_ts[lo:hi]:
                recs[t] = exp_dve(t)
            for eg in consume_at.get(g, []):
                do_exp_bank(eg)
            if g == HS - 1:
                # first-half repack can start as soon as slots 0..HS-1 exist
                for j in range(4):
                    nc.sync.dma_start(
                        out=packed[32 * j : 32 * j + HS, :],
                        in_=stage[32 * j : 32 * j + 1, 0:HS, :],
                    )
            QS = NGB // 4
            if g == 3 * QS - 1:
                for j in range(4):
                    nc.sync.dma_start(
                        out=packed[32 * j + 2 * QS : 32 * j + 3 * QS, :],
                        in_=stage[32 * j : 32 * j + 1, 2 * QS : 3 * QS, :],
                    )
        # any exp banks whose consume slot fell past the last group
        for k2, es in consume_at.items():
            if k2 >= NGA:
                for eg in es:
                    do_exp_bank(eg)

        # repack staging (t = 32j + g at partition 32j, slot g) to [t, te]:
        # each j gives one contiguous 32-row block
        QS4 = NGB // 4
        for j in range(4):
            nc.sync.dma_start(
                out=packed[32 * j + 3 * QS4 : 32 * j + 4 * QS4, :],
                in_=stage[32 * j : 32 * j + 1, 3 * QS4 : 4 * QS4, :],
            )

        # ---- softmax + context
        # scores are bounded by sum |V| (~10), so exp without max-subtraction
        # is safe in f32. accum_out gives the softmax denominator for free.
        w_s = const.tile([128, Te], BF16)
        denom = const.tile([128, 1], F32)
        nc.scalar.activation(
            w_s[:], packed[:], mybir.ActivationFunctionType.Exp, accum_out=denom[:]
        )
        rdenom = const.tile([128, 1], F32)
        nc.vector.reciprocal(rdenom[:], denom[:])

        # wT for the context matmul via PE transposes of the packed weights
        wT = const.tile([128, NB_TE, 128], BF16)
        for c in range(NB_TE):
            tp = pbig.tile([128, 128], BF16, tag="tr")
            nc.tensor.transpose(tp[:], w_s[:, 128 * c : 128 * (c + 1)], ident_bf[:])
            nc.vector.tensor_copy(wT[:, c, :], tp[:])
        ctxp = pbig.tile([128, D], F32, tag="proj")
        for c in range(NB_TE):
            nc.tensor.matmul(
                ctxp[:],
                wT[:, c, :],
                encb[:, c, :],
                start=(c == 0),
                stop=(c == NB_TE - 1),
            )
        ctx_sb = const.tile([128, D], F32)
        nc.vector.tensor_scalar_mul(ctx_sb[:], ctxp[:], rdenom[:])
        nc.sync.dma_start(out=out[:, 0:D], in_=ctx_sb[:])

        attn = const.tile([128, Te], F32)
        nc.vector.tensor_scalar_mul(attn[:], w_s[:], rdenom[:])
        nc.sync.dma_start(out=out[:, D:], in_=attn[:])


def _get_nc():
    global _NC
    if _NC is None:
        _NC = _build_core_graph()
    return _NC


def _make_in_maps(inputs):
    enc_full = np.ascontiguousarray(np.asarray(inputs["encoder_output"], np.float32))
    dec_full = np.ascontiguousarray(np.asarray(inputs["decoder_output"], np.float32))
    w_a = np.ascontiguousarray(np.asarray(inputs["W_a"], np.float32))
    u_a = np.ascontiguousarray(np.asarray(inputs["U_a"], np.float32))
    v_a = np.ascontiguousarray(np.asarray(inputs["V_a"], np.float32))
    in_maps = []
    for c in range(N_CORES):
        b, h = divmod(c, Td // TDS)
        in_maps.append(
            {
                "enc": enc_full[b],
                "dec": np.ascontiguousarray(dec_full[b, h * TDS : (h + 1) * TDS]),
                "w_a": w_a,
                "u_a": u_a,
                "v_a": v_a,
            }
        )
    return in_maps


def _assemble(results):
    ctx = np.empty((B, Td, D), np.float32)
    attn = np.empty((B, Td, Te), np.float32)
    for c in range(N_CORES):
        o = np.asarray(results[c]["out"])
        b, h = divmod(c, Td // TDS)
        ctx[b, h * TDS : (h + 1) * TDS] = o[:, :D]
        attn[b, h * TDS : (h + 1) * TDS] = o[:, D:]
    return ctx, attn


def _run(inputs, trace=False, **kwargs):
    nc = _get_nc()
    in_maps = _make_in_maps(inputs)
    res = run_bass_kernel_spmd(
        nc, in_maps, core_ids=list(range(N_CORES)), trace=trace, **kwargs
    )
    return _assemble(res.results), res


def kernel(**inputs):
    return _run(inputs)[0]


# revision 35
# speedup vs baseline: 1.0191x; 1.0002x over previous
"""Trainium2 Bass kernel for Bahdanau-style additive attention.

reference math (per batch b):
    W_enc = enc @ W_a                      # [Te, D]
    U_dec = dec @ U_a                      # [Td, D]
    e     = tanh(W_enc[None,:,:] + U_dec[:,None,:])   # [Td, Te, D]
    score = e @ V_a                        # [Td, Te]
    attn  = softmax(score, axis=-1)        # [Td, Te]
    ctx   = attn @ enc                     # [Td, D]
returns (ctx, attn).

Sharding: data-parallel over (b, Td-half) -> 8 cores, no collectives.

Per-core pipeline (d on partitions for the elementwise phase):
  - PE transposes + bf16 projections give W_encT [d'(part), te] and
    U_decT [d'(part), t]; first tanh group is te-halved so ACT starts
    as early as possible.
  - elementwise split to balance engines: most decoder positions run
    e_in = W_encT + u_t (DVE tensor_scalar, per-partition scalar) then
    tanh on ACT in batched ACTIVATEs (~0.9ns/elem, the ~105us wall);
    the rest use tanh(x) = 1 - 2/(1 + e^{2w} e^{2u}) entirely on DVE
    (fused multiply-add + reciprocal_approx_fast).
  - V-dot in ROW form: V is the 1-column stationary operand, e streams
    as the moving operand (1 col/cycle; the stationary-e alternative
    pays ~2x in LDWEIGHTS). Four positions share one PSUM bank at
    partitions {0,32,64,96} via tile_position col-groups (V broadcast
    across 32 stationary columns keeps the bank initialized), so one
    97-lane DVE copy evacuates 4 score rows at free-size cost.
  - scores repack to [t, te] with contiguous-block DMAs (bank g holds
    t = 32j + g), then exp with fused accum_out denominator, bf16 PE
    transposes + bf16 context matmul, and per-partition normalization.

Measured on trn2 (8 cores): 145.2us, rel err ~3e-3 (bf16-dominated).
"""

import numpy as np

import concourse.bass as bass
import concourse.bacc as bacc
import concourse.tile as tile
from concourse import mybir
from concourse.bass_utils import run_bass_kernel_spmd
from concourse.masks import make_identity

F32 = mybir.dt.float32
BF16 = mybir.dt.bfloat16

B, Te, Td, D = 4, 512, 256, 256
N_CORES = 8
TDS = Td * B // N_CORES          # 128 decoder positions per core
NB_TE = Te // 128                # 4 te blocks
NB_D = D // 128                  # 2 d blocks
G = 4                            # decoder positions per tanh batch
# ACT/DVE split: t < TANH_T run tanh on ACT; the rest use
# tanh(x) = 1 - 2/(1 + e^{2w} e^{2u}) with the fused multiply-add +
# reciprocal on DVE, balancing the two engines (ACT is otherwise the
# ~134us bottleneck while DVE sits at ~50us).
TANH_T = 116
NGA = TANH_T // G
WORK_BUFS = 4

_NC = None


def _build_core_graph():
    # Bacc (not plain Bass): its compile() runs move_matmul_waits_to_ldweights
    # + generate_event_semaphores, which legalize to <=1 sync wait per
    # instruction (walrus rejects multi-wait Matmults with "Too many sync
    # wait commands").
    nc = bacc.Bacc(trn_type="TRN2")
    enc = nc.dram_tensor("enc", [Te, D], F32, kind="ExternalInput")
    dec = nc.dram_tensor("dec", [TDS, D], F32, kind="ExternalInput")
    w_a = nc.dram_tensor("w_a", [D, D], F32, kind="ExternalInput")
    u_a = nc.dram_tensor("u_a", [D, D], F32, kind="ExternalInput")
    v_a = nc.dram_tensor("v_a", [D, 1], F32, kind="ExternalInput")
    out = nc.dram_tensor("out", [TDS, D + Te], F32, kind="ExternalOutput")

    with tile.TileContext(nc) as tc:
        _body(tc, enc, dec, w_a, u_a, v_a, out)
    nc.compile()
    return nc


def _body(tc, enc, dec, w_a, u_a, v_a, out):
    nc = tc.nc
    from contextlib import ExitStack

    with ExitStack() as ctx:
        const = ctx.enter_context(tc.tile_pool(name="const", bufs=1))
        work = ctx.enter_context(tc.tile_pool(name="work", bufs=WORK_BUFS))
        pbig = ctx.enter_context(
            tc.tile_pool(name="pbig", bufs=2, space=bass.MemorySpace.PSUM)
        )
        pbank = ctx.enter_context(
            tc.tile_pool(name="pbank", bufs=4, space=bass.MemorySpace.PSUM)
        )
        recpool = ctx.enter_context(
            tc.tile_pool(name="recpool", bufs=max(1, TDS - TANH_T))
        )

        ident = const.tile([128, 128], F32)
        make_identity(nc, ident[:])

        # ---- inputs -> SBUF
        # enc_ext carries a leading ones column per te-block so the context
        # matmul also produces the softmax denominator in column 0.
        enc_ext = const.tile([128, NB_TE, 1 + D], F32)
        nc.vector.memset(enc_ext[:, :, 0:1], 1.0)
        enc_r = enc.rearrange("(n p) d -> p n d", p=128)
        for n in range(NB_TE):
            # split across DMA queues so the transposes can start sooner
            nc.sync.dma_start(out=enc_ext[:, n, 1:], in_=enc_r[:, n, :])
        dec_sb = const.tile([128, D], F32)
        nc.sync.dma_start(out=dec_sb[:], in_=dec[:, :])
        wa_sb = const.tile([128, NB_D, D], F32)
        nc.sync.dma_start(out=wa_sb[:], in_=w_a.rearrange("(k p) e -> p k e", p=128))
        ua_sb = const.tile([128, NB_D, D], F32)
        nc.sync.dma_start(out=ua_sb[:], in_=u_a.rearrange("(k p) e -> p k e", p=128))
        v_sb = const.tile([128, NB_D], F32)
        nc.sync.dma_start(out=v_sb[:], in_=v_a.rearrange("(k p) o -> p (k o)", p=128))
        v_bf = const.tile([128, NB_D], BF16)
        nc.vector.tensor_copy(v_bf[:], v_sb[:])

        # ---- bf16 working copies (prep-latency: bf16 matmuls stream 4x
        # faster than f32, and the first tanh group is chained behind the
        # enc DMA -> transpose -> projection path)
        ident_bf = const.tile([128, 128], BF16)
        nc.vector.tensor_copy(ident_bf[:], ident[:])
        encb = const.tile([128, NB_TE, D], BF16)
        for h in range(2):
            nc.vector.tensor_copy(
                encb[:, 2 * h : 2 * (h + 1), :], enc_ext[:, 2 * h : 2 * (h + 1), 1:]
            )
        wab = const.tile([128, NB_D, D], BF16)
        nc.vector.tensor_copy(wab[:], wa_sb[:])
        uab = const.tile([128, NB_D, D], BF16)
        nc.vector.tensor_copy(uab[:], ua_sb[:])
        decb = const.tile([128, D], BF16)
        nc.vector.tensor_copy(decb[:], dec_sb[:])

        # ---- encT [d(part), te], decT [d(part), t] via PE transposes (bf16)
        encT = const.tile([128, NB_D, Te], BF16)
        for n in range(NB_TE):
            for k in range(NB_D):
                tp = pbig.tile([128, 128], BF16, tag="tr")
                nc.tensor.transpose(
                    tp[:], encb[:, n, 128 * k : 128 * (k + 1)], ident_bf[:]
                )
                nc.vector.tensor_copy(encT[:, k, 128 * n : 128 * (n + 1)], tp[:])
        decT = const.tile([128, NB_D, TDS], BF16)
        for k in range(NB_D):
            tp = pbig.tile([128, 128], BF16, tag="tr")
            nc.tensor.transpose(tp[:], decb[:, 128 * k : 128 * (k + 1)], ident_bf[:])
            nc.vector.tensor_copy(decT[:, k, :], tp[:])

        # ---- projections (PE, bf16 operands, f32 PSUM), te-halved so the
        # first half of w_encT lands as early as possible
        w_encT = const.tile([128, NB_D, Te], BF16)
        HT = Te // 2
        for h in range(2):
            for j in range(NB_D):
                pj = pbig.tile([128, HT], F32, tag="proj")
                for k in range(NB_D):
                    nc.tensor.matmul(
                        pj[:],
                        wab[:, k, 128 * j : 128 * (j + 1)],
                        encT[:, k, HT * h : HT * (h + 1)],
                        start=(k == 0),
                        stop=(k == NB_D - 1),
                    )
                nc.scalar.copy(w_encT[:, j, HT * h : HT * (h + 1)], pj[:])
        # scalar operand of tensor_scalar must be f32 -> keep u_decT f32
        u_decT = const.tile([128, NB_D, TDS], F32)
        for j in range(NB_D):
            pj = pbig.tile([128, HT], F32, tag="proj")
            for k in range(NB_D):
                nc.tensor.matmul(
                    pj[:, :TDS],
                    uab[:, k, 128 * j : 128 * (j + 1)],
                    decT[:, k, :],
                    start=(k == 0),
                    stop=(k == NB_D - 1),
                )
            nc.scalar.copy(u_decT[:, j, :], pj[:, :TDS])

        # P2 for the DVE tanh-identity path, sourced from bf16 w_encT
        # (emitted after the first tanh group so it does not delay it)
        p2 = const.tile([128, NB_D, Te], F32)
                # Q2[j][d', t] = exp(2 * u_decT) (f32; tensor_scalar scalars must be
        # f32). -2*V feeds the identity-path matmuls: score' = -2 * sum V*r
        # differs from the true score by a per-t constant, which softmax
        # normalizes away.
        q2 = const.tile([128, NB_D, TDS], F32)
        v2m = const.tile([128, NB_D], F32)
        nc.vector.tensor_scalar_mul(v2m[:], v_sb[:], -2.0)

        # ---- main loop (v5): row-form V-dot.
        # The v4 stationary-e form streamed all of e through the PE weight
        # port (LDWEIGHTS ~196us on HW). Row form keeps V stationary (1-col
        # LDW) and streams e as the moving operand (1 col/cycle @2.4GHz,
        # ~55us). Scores come out as [1, 512] rows; we pack 4 decoder
        # positions per PSUM bank at partitions {0,32,64,96} via
        # tile_position col-groups, evacuate each bank with ONE 97-lane DVE
        # copy (cost is free-size-bound, so 4 rows for the price of one),
        # and repack to [t, te] later with 4 strided DMAs.
        NGB = TDS // 4  # bank-groups; bank g holds t = 32j + g at partition 32j
        stage = const.tile([128, NGB, Te], BF16)

        def vdot_row(bank, j, src_e, vcol, lo=0, hi=Te):
            # M=32 with V broadcast across the stationary columns: all 32
            # partitions of the col-group get (duplicate) score rows, which
            # keeps the bank fully initialized for the 97-lane evacuation
            # copy. Streaming cost is N-bound, so the extra rows are free.
            for k in range(NB_D):
                nc.tensor.matmul(
                    bank[32 * j : 32 * j + 32, lo:hi],
                    vcol[:, k : k + 1].to_broadcast((128, 32)),
                    src_e[:, k, lo:hi],
                    start=(k == 0),
                    stop=(k == NB_D - 1),
                    tile_position=(0, 32 * j),
                )

        def exp_dve(t):
            den = work.tile([128, NB_D, Te], F32, tag="den")
            for k in range(NB_D):
                nc.vector.tensor_scalar(
                    den[:, k, :],
                    p2[:, k, :],
                    q2[:, k, t : t + 1],
                    1.0,
                    op0=mybir.AluOpType.mult,
                    op1=mybir.AluOpType.add,
                )
            # den = 1 + e^{2(w+u)} in [1, ~1e6]: safely inside
            # reciprocal_approx_fast's domain.
            rec = recpool.tile([128, NB_D, Te], F32, tag="rec")
            nc.vector.reciprocal_approx_fast(out=rec[:], in_=den[:])
            return rec

        packed = const.tile([128, Te], BF16)
        HS = NGB // 2
        exp_gs = list(range(NGA, NGB))
        exp_ts = [32 * j + g for g in exp_gs for j in range(4)]
        recs = {}
        # produce all identity-path reciprocals during the first ~2/3 of the
        # tanh groups, and consume each exp bank mid-loop so its f32 V-dots
        # (4 cyc/col on PE) hide in PE idle time instead of extending the
        # tail.
        DVE_GS = max(1, NGA - 4)
        consume_at = {}
        for idx, g in enumerate(exp_gs):
            consume_at.setdefault(NGA - 8 + 3 * idx, []).append(g)

        def do_exp_bank(g):
            bank = pbank.tile([128, Te], F32, tag="bank")
            for j in range(4):
                vdot_row(bank, j, recs[32 * j + g], v2m)
            nc.vector.tensor_copy(stage[0:97, g, :], bank[0:97, :])

        for g in range(NGA):
            e_in = work.tile([128, G, NB_D, Te], BF16, tag="e_in")
            e_tanh = work.tile([128, G, NB_D, Te], BF16, tag="e_tanh")
            if g == 0:
                # First group split into te-halves so the first TANH starts
                # as soon as the first half of w_encT exists; P2/Q2 for the
                # identity path are emitted right after (ACT stays the
                # bottleneck either way, but they must not delay this tanh).
                for h in range(2):
                    for i in range(G):
                        t = 32 * i + g
                        for k in range(NB_D):
                            nc.vector.tensor_scalar_add(
                                e_in[:, i, k, HT * h : HT * (h + 1)],
                                w_encT[:, k, HT * h : HT * (h + 1)],
                                u_decT[:, k, t : t + 1],
                            )
                    nc.scalar.activation(
                        e_in[:, :, :, HT * h : HT * (h + 1)],
                        e_in[:, :, :, HT * h : HT * (h + 1)],
                        mybir.ActivationFunctionType.Tanh,
                    )
                e_tanh = e_in
                nc.scalar.activation(
                    p2[:], w_encT[:], mybir.ActivationFunctionType.Exp, scale=2.0
                )
                nc.scalar.activation(
                    q2[:], u_decT[:], mybir.ActivationFunctionType.Exp, scale=2.0
                )
            elif g == NGA - 1:
                # last group te-halved too: the kernel tail chains off this
                # group's tanh -> V-dots -> evacuation, so finishing the
                # first half early shortens the drain.
                bank = pbank.tile([128, Te], F32, tag="bank")
                for h in range(2):
                    for i in range(G):
                        t = 32 * i + g
                        for k in range(NB_D):
                            nc.vector.tensor_scalar_add(
                                e_in[:, i, k, HT * h : HT * (h + 1)],
                                w_encT[:, k, HT * h : HT * (h + 1)],
                                u_decT[:, k, t : t + 1],
                            )
                    nc.scalar.activation(
                        e_in[:, :, :, HT * h : HT * (h + 1)],
                        e_in[:, :, :, HT * h : HT * (h + 1)],
                        mybir.ActivationFunctionType.Tanh,
                    )
                    for i in range(G):
                        vdot_row(
                            bank, i, e_in[:, i], v_bf, HT * h, HT * (h + 1)
                        )
                nc.vector.tensor_copy(stage[0:97, g, :], bank[0:97, :])
            else:
                for i in range(G):
                    t = 32 * i + g
                    for k in range(NB_D):
                        nc.vector.tensor_scalar_add(
                            e_in[:, i, k, :], w_encT[:, k, :], u_decT[:, k, t : t + 1]
                        )
                nc.scalar.activation(
                    e_tanh[:], e_in[:], mybir.ActivationFunctionType.Tanh
                )
            if g != NGA - 1:
                bank = pbank.tile([128, Te], F32, tag="bank")
                for i in range(G):
                    vdot_row(bank, i, e_tanh[:, i], v_bf)
                nc.vector.tensor_copy(stage[0:97, g, :], bank[0:97, :])
            lo = len(exp_ts) * g // DVE_GS
            hi = len(exp_ts) * (g + 1) // DVE_GS
            for t in exp_ts[lo:hi]:
                recs[t] = exp_dve(t)
            for eg in consume_at.get(g, []):
                do_exp_bank(eg)
            if g == HS - 1:
                # first-half repack can start as soon as slots 0..HS-1 exist
                for j in range(4):
                    nc.sync.dma_start(
                        out=packed[32 * j : 32 * j + HS, :],
                        in_=stage[32 * j : 32 * j + 1, 0:HS, :],
                    )
            QS = NGB // 4
            if g == 3 * QS - 1:
                for j in range(4):
                    nc.sync.dma_start(
                        out=packed[32 * j + 2 * QS : 32 * j + 3 * QS, :],
                        in_=stage[32 * j : 32 * j + 1, 2 * QS : 3 * QS, :],
                    )
        # any exp banks whose consume slot fell past the last group
        for k2, es in consume_at.items():
            if k2 >= NGA:
                for eg in es:
                    do_exp_bank(eg)

        # repack staging (t = 32j + g at partition 32j, slot g) to [t, te]:
        # each j gives one contiguous 32-row block
        QS4 = NGB // 4
        for j in range(4):
            nc.sync.dma_start(
                out=packed[32 * j + 3 * QS4 : 32 * j + 4 * QS4, :],
                in_=stage[32 * j : 32 * j + 1, 3 * QS4 : 4 * QS4, :],
            )

        # ---- softmax + context
        # scores are bounded by sum |V| (~10), so exp without max-subtraction
        # is safe in f32. accum_out gives the softmax denominator for free.
        w_s = const.tile([128, Te], BF16)
        denom = const.tile([128, 1], F32)
        nc.scalar.activation(
            w_s[:], packed[:], mybir.ActivationFunctionType.Exp, accum_out=denom[:]
        )
        rdenom = const.tile([128, 1], F32)
        nc.vector.reciprocal(rdenom[:], denom[:])

        # wT for the context matmul via PE transposes of the packed weights
        wT = const.tile([128, NB_TE, 128], BF16)
        for c in range(NB_TE):
            tp = pbig.tile([128, 128], BF16, tag="tr")
            nc.tensor.transpose(tp[:], w_s[:, 128 * c : 128 * (c + 1)], ident_bf[:])
            nc.vector.tensor_copy(wT[:, c, :], tp[:])
        ctxp = pbig.tile([128, D], F32, tag="proj")
        for c in range(NB_TE):
            nc.tensor.matmul(
                ctxp[:],
                wT[:, c, :],
                encb[:, c, :],
                start=(c == 0),
                stop=(c == NB_TE - 1),
            )
        ctx_sb = const.tile([128, D], F32)
        nc.vector.tensor_scalar_mul(ctx_sb[:], ctxp[:], rdenom[:])
        nc.sync.dma_start(out=out[:, 0:D], in_=ctx_sb[:])

        attn = const.tile([128, Te], F32)
        nc.vector.tensor_scalar_mul(attn[:], w_s[:], rdenom[:])
        nc.sync.dma_start(out=out[:, D:], in_=attn[:])


def _get_nc():
    global _NC
    if _NC is None:
        _NC = _build_core_graph()
    return _NC


def _make_in_maps(inputs):
    enc_full = np.ascontiguousarray(np.asarray(inputs["encoder_output"], np.float32))
    dec_full = np.ascontiguousarray(np.asarray(inputs["decoder_output"], np.float32))
    w_a = np.ascontiguousarray(np.asarray(inputs["W_a"], np.float32))
    u_a = np.ascontiguousarray(np.asarray(inputs["U_a"], np.float32))
    v_a = np.ascontiguousarray(np.asarray(inputs["V_a"], np.float32))
    in_maps = []
    for c in range(N_CORES):
        b, h = divmod(c, Td // TDS)
        in_maps.append(
            {
                "enc": enc_full[b],
                "dec": np.ascontiguousarray(dec_full[b, h * TDS : (h + 1) * TDS]),
                "w_a": w_a,
                "u_a": u_a,
                "v_a": v_a,
            }
        )
    return in_maps


def _assemble(results):
    ctx = np.empty((B, Td, D), np.float32)
    attn = np.empty((B, Td, Te), np.float32)
    for c in range(N_CORES):
        o = np.asarray(results[c]["out"])
        b, h = divmod(c, Td // TDS)
        ctx[b, h * TDS : (h + 1) * TDS] = o[:, :D]
        attn[b, h * TDS : (h + 1) * TDS] = o[:, D:]
    return ctx, attn


def _run(inputs, trace=False, **kwargs):
    nc = _get_nc()
    in_maps = _make_in_maps(inputs)
    res = run_bass_kernel_spmd(
        nc, in_maps, core_ids=list(range(N_CORES)), trace=trace, **kwargs
    )
    return _assemble(res.results), res


def kernel(**inputs):
    return _run(inputs)[0]
